# revision 1
# baseline (speedup 1.0000x reference)
"""Multi-head attention (B*H=64, S=2048, D=64) on 8 Trainium2 cores.

Sharding: 64 heads -> 8 per core (head-parallel, no communication).

Per-core kernel (heads processed in pairs A/B stacked on SBUF partition
halves 0:64 / 64:128):
  - prep (all pairs upfront): q/k are DMA'd with a 32x32-block-permuted
    access pattern, then a single DVE 32x32 block-transpose per tensor
    yields stacked Q^T/K^T [128(dA|dB), 2048] in natural q order, rounded
    to float32r.  V is loaded naturally and cast to bf16 with an appended
    ones column (so the PV matmul also produces the softmax denominator).
  - main loop per pair x (q-chunk 512) x (16 k-tiles of 128):
      S^T[k,q] = K Q^T   -- two row-packed float32r matmuls
                            (tile_position (0,0)/(64,0)), concurrent on PE
      P^T = exp(S^T)     -- bf16; exact exp on ACT for most k-tiles,
                            Schraudolph int16 bit-trick on DVE for the rest
      O^T[d+1,q] += V_aug^T P^T -- bf16 matmul, PSUM-accumulated;
                            row 64 accumulates the denominator Z
  - epilogue per (pair, head, q-chunk): copy O^T to SBUF, PE-transpose
    back to [q, d+1], DVE reciprocal of the Z column, scale, DMA out.
"""

import os

import numpy as np

import concourse.bass as bass
import concourse.mybir as mybir
import concourse.tile as tile
from concourse import bacc
from concourse.bass_utils import run_bass_kernel_spmd
from concourse.masks import make_identity

B, S, D = 64, 2048, 64
NCORES = 8
H = B // NCORES  # heads per core
P = 128  # partitions
KT = S // P  # 16 k-tiles
QC = 512  # q-chunk
NQC = S // QC  # 4 q-chunks
NPAIR = H // 2  # head pairs per core

F32 = mybir.dt.float32
F32R = mybir.dt.float32r
BF16 = mybir.dt.bfloat16
I16 = mybir.dt.int16
F16 = mybir.dt.float16

# Number of k-tiles (of 16) whose exp runs on DVE via the Schraudolph bit
# trick (approximate, ~2% rms per weight); the rest run exact exp on ACT.
DVE_EXP_KT = int(os.environ.get("BASS_ATTN_DVE_EXP_KT", "5"))
# k-tiles spread evenly so ACT and DVE exp work interleaves/overlaps
_DVE_KTS = set()
if DVE_EXP_KT > 0:
    _DVE_KTS = {round((i + 0.5) * 16 / DVE_EXP_KT) % 16 for i in range(DVE_EXP_KT)}

# Schraudolph constants for bf16 exp via int16 bit pattern:
#   i = round_int16(x * 2^7/ln2 + b);  exp(x) ~= bitcast_bf16(i)
# b calibrated for round-to-nearest convert (max rel err ~3.3%).
_SCH_A = float(128.0 / np.log(2.0))
_SCH_B = float(os.environ.get("BASS_ATTN_SCH_B", "16250.5"))


def build_attention_nc() -> bass.Bass:
    nc = bacc.Bacc()
    q_d = nc.declare_dram_parameter("q", [H, S, D], F32, isOutput=False)
    k_d = nc.declare_dram_parameter("k", [H, S, D], F32, isOutput=False)
    v_d = nc.declare_dram_parameter("v", [H, S, D], F32, isOutput=False)
    o_d = nc.declare_dram_parameter("out", [H, S, D], F32, isOutput=True)

    # 32x32-block-permuted views for the transpose loads:
    #   staging[32a + i, 32b + j] = x[32b + i, 32a + j]
    q_bp = q_d.rearrange("h (b i) (a j) -> h a i b j", i=32, j=32)
    k_bp = k_d.rearrange("h (b i) (a j) -> h a i b j", i=32, j=32)
    # natural views: row = t*128 + m (k index), row = g*512 + c*128 + p (q)
    v_v = v_d.rearrange("h (t p) d -> h p t d", p=P)
    o_v = o_d.rearrange("h (g c p) d -> h p g c d", c=4, p=P)

    with tile.TileContext(nc) as tc:
        with (
            tc.tile_pool(name="consts", bufs=1) as consts,
            tc.tile_pool(name="stage", bufs=4) as stage,
            tc.tile_pool(name="qk_t", bufs=16) as qkt_pool,
            tc.tile_pool(name="vpool", bufs=4) as vpool,
            tc.tile_pool(name="ppool", bufs=6) as ppool,
            tc.tile_pool(name="osb", bufs=4) as osb_pool,
            tc.tile_pool(name="outsb", bufs=4) as outsb_pool,
            tc.tile_pool(name="rz", bufs=4) as rz_pool,
            tc.tile_pool(name="spsum", bufs=3, space="PSUM") as spsum,
            tc.tile_pool(name="oep", bufs=2, space="PSUM") as opsum,
        ):
            ident = consts.tile([D + 1, D + 1], F32)
            make_identity(nc, ident[:])
            ones16 = consts.tile([P, KT], F32)
            nc.vector.memset(ones16[:], 1.0)

            for pair in range(NPAIR):
                h_a, h_b = 2 * pair, 2 * pair + 1

                # ---------------- prep ----------------
                # K first (the kt loop sweeps all K chunks before moving to
                # the next q chunk), per-512-column chunks for fine-grained
                # pipelining of DMA -> block-transpose -> fp16 round.
                qkt = {"q": [], "k": []}
                for fc in range(4):
                    for name, src in (("k", k_bp), ("q", q_bp)):
                        st = stage.tile([P, QC], F32, tag="stage")
                        for hh, pb in ((h_a, 0), (h_b, 2)):
                            for a in range(2):
                                c = pb + a
                                nc.sync.dma_start(
                                    out=st[32 * c : 32 * c + 32, :].rearrange(
                                        "i (b j) -> i b j", j=32
                                    ),
                                    in_=src[hh, a, :, 16 * fc : 16 * fc + 16, :],
                                )
                        st2 = stage.tile([P, QC], F32, tag="stage2")
                        nc.vector.transpose(st2[:], st[:])
                        ch = qkt_pool.tile([P, QC], F16, tag="qkT")
                        nc.vector.tensor_copy(ch[:], st2[:])
                        qkt[name].append(ch)

                v_aug = {}
                for hh, part in ((h_a, 0), (h_b, 1)):
                    vst = stage.tile([P, KT, D], F32, tag="vstage")
                    nc.sync.dma_start(out=vst[:], in_=v_v[hh])
                    va = vpool.tile([P, KT, D + 1], BF16, tag="v")
                    nc.vector.tensor_copy(va[:, :, 0:D], vst[:])
                    nc.vector.tensor_copy(va[:, :, D], ones16[:])
                    v_aug[part] = va

                # ---------------- main ----------------
                kT = qkt["k"]
                qT = qkt["q"]
                for g in range(NQC):
                    o_ps_a = opsum.tile([D + 1, QC], F32, tag="oep")
                    o_ps_b = opsum.tile([D + 1, QC], F32, tag="oep")
                    o_ps = {0: o_ps_a, 1: o_ps_b}

                    def emit_qkt(kt):
                        s_ps = spsum.tile([P, 2, QC], F32, tag="s")
                        k_ch = kT[kt // 4]
                        k_sl = slice((kt % 4) * P, (kt % 4 + 1) * P)
                        for part, base in ((0, 0), (1, 64)):
                            nc.tensor.matmul(
                                s_ps[:, part, :],
                                k_ch[base : base + 64, k_sl],
                                qT[g][base : base + 64, :],
                                tile_position=(base, 0),
                            )
                        return s_ps

                    # software-pipelined by two k-tiles: QK^T runs ahead of
                    # PV/exp so the PE works while exp(kt) runs.
                    s_tiles = {i: emit_qkt(i) for i in range(3)}
                    for kt in range(KT):
                        s_ps = s_tiles.pop(kt)
                        p_sb = ppool.tile([P, 2, QC], BF16, tag="p")
                        if kt in _DVE_KTS:
                            nc.vector.tensor_scalar(
                                out=p_sb[:].bitcast(I16),
                                in0=s_ps[:],
                                scalar1=_SCH_A,
                                scalar2=_SCH_B,
                                op0=mybir.AluOpType.mult,
                                op1=mybir.AluOpType.add,
                            )
                        else:
                            nc.scalar.activation(
                                p_sb[:], s_ps[:], mybir.ActivationFunctionType.Exp
                            )
                        for part in (0, 1):
                            nc.tensor.matmul(
                                o_ps[part][:],
                                v_aug[part][:, kt, :],
                                p_sb[:, part, :],
                                start=(kt == 0),
                                stop=(kt == KT - 1),
                            )
                        if kt + 3 < KT:
                            s_tiles[kt + 3] = emit_qkt(kt + 3)

                    # ---------------- epilogue ----------------
                    for part, hh in ((0, h_a), (1, h_b)):
                        o_sb = osb_pool.tile([D + 1, QC], F32, tag="ot")
                        nc.scalar.copy(o_sb[:], o_ps[part][:])
                        ep = opsum.tile([P, 4, D + 1], F32, tag="oep")
                        for c in range(4):
                            nc.tensor.transpose(
                                ep[:, c, :],
                                o_sb[:, c * P : (c + 1) * P],
                                ident[:],
                            )
                        rz = rz_pool.tile([P, 4], F32, tag="rz")
                        nc.vector.reciprocal(rz[:], ep[:, :, D])
                        out_sb = outsb_pool.tile([P, 4, D], F32, tag="out")
                        for c in range(4):
                            nc.vector.tensor_scalar(
                                out=out_sb[:, c, :],
                                in0=ep[:, c, 0:D],
                                scalar1=rz[:, c : c + 1],
                                scalar2=None,
                                op0=mybir.AluOpType.mult,
                            )
                        nc.sync.dma_start(out=o_v[hh, :, g, :, :], in_=out_sb[:])
    nc.finalize()
    return nc


_NC_CACHE = None


def _get_nc():
    global _NC_CACHE
    if _NC_CACHE is None:
        _NC_CACHE = build_attention_nc()
    return _NC_CACHE


def kernel(q: np.ndarray, k: np.ndarray, v: np.ndarray) -> np.ndarray:
    q = np.asarray(q, dtype=np.float32)
    k = np.asarray(k, dtype=np.float32)
    v = np.asarray(v, dtype=np.float32)
    nc = _get_nc()
    in_maps = [
        {
            "q": np.ascontiguousarray(q[c * H : (c + 1) * H]),
            "k": np.ascontiguousarray(k[c * H : (c + 1) * H]),
            "v": np.ascontiguousarray(v[c * H : (c + 1) * H]),
        }
        for c in range(NCORES)
    ]
    res = run_bass_kernel_spmd(nc, in_maps, list(range(NCORES)))
    return np.concatenate([res.results[c]["out"] for c in range(NCORES)], axis=0)

